# revision 34
# baseline (speedup 1.0000x reference)
"""2-layer GCN (PyG GCNConv semantics) on 8 Trainium2 NeuronCores.

Math: out = A_hat @ relu(A_hat @ X @ W1 + b1) @ W2 + b2,
      A_hat = D^-1/2 (A + I) D^-1/2, D = in-degree + 1.

Folding the symmetric norm into per-node scales:
  g = dinv * (H @ W);  s[c] = sum_{e:src->c} g[src] + g[c];  out = dinv*s + b

Sharding: destination-node ranges (12500 nodes/core). Each core:
  Phase A: compute the FULL g1 = dinv*(X@W1) table redundantly (PE idle
           anyway), stored BF16 and TRANSPOSED ([128, XW*F1] partition-major)
           so the writes are large contiguous descriptors.
  Phase B: batched dma_gather of edge-source rows (edges laid out in uniform
           (source-group, dest-window) cells so one SPMD program serves all
           cores), segment-sum via one-hot is_equal + bf16 matmul into PSUM
           per 128-dest window, accumulated into an fp32 SBUF accumulator.
           Self-loop contributions are NOT gathered: the accumulator is
           initialized from the core's own contiguous block of the transposed
           table (one register-offset SWDGE DMA with bf16->f32 cast).
  Phase C: finalize layer 1 per window, compute g2 shard (bf16, padded to
           128 feature columns so gather elements stay 256B-aligned); copy
           the fp32 g2 into the layer-2 accumulator (self-loop init for L2).
  AllGather g2 shards -> full g2 table.
  Phase D: layer-2 aggregation (same machinery; matmul consumes msg[:, :64]).
  Phase E: finalize layer 2, write output shard.

Gather descriptors are the dominant cost: ~250k edge slots per core per
layer, one 256B descriptor each. Descriptor generation runs on Q7 core pair
(2q, 2q+1) for SWDGE queue q, so rotating gather calls over 4 queues
parallelizes the emit 4x. The int16 gather-index limit (32767) forces
grouping edges by 25088-row source ranges; trailing chunk padding inside
each (window,group) cell uses src_local=0 and dest_local=-1 (one-hot zero).
"""

import os
import numpy as np
import ml_dtypes

import concourse.bass as bass
from concourse import bacc
import concourse.mybir as mybir
import concourse.tile as tile
from concourse import bass_utils

F32 = mybir.dt.float32
BF16 = mybir.dt.bfloat16
I16 = mybir.dt.int16
NPBF16 = ml_dtypes.bfloat16

NCORES = 8
NGROUPS = 4
CALL = int(os.environ.get('GCN_CALL', '4096'))   # gather-call size in edge slots
NQUEUES = int(os.environ.get('GCN_NQUEUES', '4'))
DMA_SCRATCH = int(os.environ.get('GCN_DMA_SCRATCH', '32768'))
SINGLE_PACKET = bool(int(os.environ.get('GCN_SP', '0')))
PHA_WIN = 16         # windows per phase-A iteration


def _cfg(n_nodes, f1, f2):
    shard = n_nodes // NCORES
    nw = (shard + 127) // 128
    shard_pad = nw * 128
    rows = NCORES * shard_pad          # padded table rows (both layers)
    gw = -(-rows // NGROUPS)
    gw = ((gw + 127) // 128) * 128     # group width, multiple of 128
    assert gw <= 32768, gw
    return dict(N=n_nodes, F1=f1, F2=f2, SHARD=shard, NW=nw,
                SHARD_PAD=shard_pad, ROWS=rows, GW=gw,
                XW=rows // 128)


def _layout(cnt_kwg, nw):
    """cnt_kwg: [NCORES, NW, NGROUPS] edge counts. Returns uniform cell
    capacities C[g][w] (multiples of 128), cell slot offsets, total slots S,
    gather call list [(g, slot0, nslots)], and per-chunk cell bookkeeping."""
    cmax = cnt_kwg.max(axis=0)                      # [NW, NGROUPS]
    C = ((cmax + 127) // 128) * 128                 # [NW, NGROUPS]
    cell_off = np.zeros((NGROUPS, nw), np.int64)
    off = 0
    calls = []
    cells = []            # list of (g, w, chunk0, nchunks)
    chunk_cell = []       # per global chunk -> cell index
    for g in range(NGROUPS):
        g0 = off
        for w in range(nw):
            cell_off[g, w] = off
            c = int(C[w, g])
            if c > 0:
                ci = len(cells)
                cells.append((g, w, off // 128, c // 128))
                chunk_cell.extend([ci] * (c // 128))
            off += c
        glen = off - g0
        s = g0
        while s < off:
            ns = min(CALL, off - s)
            calls.append((g, s, ns))
            s += ns
        assert glen % 128 == 0
    return C, cell_off, off, calls, cells, np.asarray(chunk_cell)


def _pack_edges(slot, lidx, dl, S):
    """Build the wrapped int16 index tensor and packed dest-local tensor."""
    src = np.zeros(S, np.int16)
    dst = np.full(S, -1.0, np.float32)
    src[slot] = lidx.astype(np.int16)
    dst[slot] = dl.astype(np.float32)
    idx_w = np.tile(src.reshape(S // 16, 16).T, (8, 1)).copy()    # [128, S/16]
    dst_p = dst.reshape(S // 128, 128).T.astype(NPBF16)           # [128, S/128]
    return idx_w, dst_p


def _prep(x, edge_index, W1, b1, W2, b2, cfg):
    N, SHARD, NW, SHARD_PAD, GW = (cfg["N"], cfg["SHARD"], cfg["NW"],
                                   cfg["SHARD_PAD"], cfg["GW"])
    ROWS, F1, F2, XW = cfg["ROWS"], cfg["F1"], cfg["F2"], cfg["XW"]

    row = np.asarray(edge_index[0]).astype(np.int64)
    col = np.asarray(edge_index[1]).astype(np.int64)
    deg = (np.bincount(col, minlength=N) + 1).astype(np.float32)
    dinv = (1.0 / np.sqrt(deg)).astype(np.float32)

    # balanced destination assignment: nodes sorted by in-degree, dealt
    # round-robin over cores, filling windows sequentially. Equalizes per
    # (window, source-group) cell counts across cores, shrinking the
    # max-over-cores cell padding (~5% fewer gather slots).
    indeg = np.bincount(col, minlength=N)
    order = np.argsort(-indeg, kind='stable')
    ranks = np.empty(N, np.int64)
    ranks[order] = np.arange(N)
    k_of = ranks % NCORES
    pos = ranks // NCORES
    w_of = pos >> 7
    dl_of = pos & 127
    rho_of = k_of * SHARD_PAD + pos  # pos < SHARD <= SHARD_PAD

    # padded node order: node n -> padded row rho_of[n]
    dinv_pad = np.zeros(ROWS, np.float32)
    xT_pad = np.zeros((F1, ROWS), NPBF16)
    xT = np.ascontiguousarray(np.asarray(x, np.float32).T.astype(NPBF16))
    dinv_pad[rho_of] = dinv
    xT_pad[:, rho_of] = xT

    k_arr = k_of[col]
    wl = w_of[col]
    dl = dl_of[col]

    # layer-1 source ids: sigma1 over the transposed padded g1 table
    # (padded row rho -> table row (rho%128)*XW + rho//128).
    rho = rho_of[row]
    src1 = (rho % 128) * XW + rho // 128
    # layer-2: g2f is a stack of locally-transposed shards [NCORES*128, NW*128]
    # (bf16, feature-padded); node at (k, w, p) -> row (k*128 + p)*NW + w.
    src2 = (k_of[row] * 128 + dl_of[row]) * NW + w_of[row]

    layers = []
    for src in (src1, src2):
        g = src // GW
        lidx = src - g * GW
        key = (k_arr * NW + wl) * NGROUPS + g
        cnt = np.bincount(key, minlength=NCORES * NW * NGROUPS)
        cnt = cnt.reshape(NCORES, NW, NGROUPS)
        C, cell_off, S, calls, cells, chunk_cell = _layout(cnt, NW)
        order = np.lexsort((wl, g, k_arr))
        ks, gs, ws = k_arr[order], g[order], wl[order]
        keys = (ks * NGROUPS + gs) * NW + ws
        starts = np.r_[0, np.nonzero(np.diff(keys))[0] + 1]
        run_id = np.zeros(len(keys), np.int64)
        run_id[starts[1:]] = 1
        run_id = np.cumsum(run_id)
        within = np.arange(len(keys)) - starts[run_id]
        slot = cell_off[gs, ws] + within
        idx_list, dst_list = [], []
        for k in range(NCORES):
            m = ks == k
            iw, dp = _pack_edges(slot[m], lidx[order][m], dl[order][m], S)
            idx_list.append(iw)
            dst_list.append(dp)
        layers.append(dict(S=S, calls=calls, cells=cells,
                           chunk_cell=chunk_cell, idx=idx_list, dst=dst_list))

    dinvA = dinv_pad.reshape(XW, 128).T.copy()                 # [128, XW]
    dinvO = [dinv_pad[k * SHARD_PAD:(k + 1) * SHARD_PAD].reshape(NW, 128).T.copy()
             for k in range(NCORES)]
    iota = np.tile(np.arange(128, dtype=np.float32), (128, 32, 1)).astype(NPBF16)
    ident = np.eye(128, dtype=np.float32)
    b1b = np.tile(np.asarray(b1, np.float32), (128, 1))
    b2b = np.tile(np.asarray(b2, np.float32), (128, 1))

    in_maps = []
    for k in range(NCORES):
        in_maps.append({
            "xT": xT_pad, "W1": np.asarray(W1, np.float32).astype(NPBF16),
            "W2": np.asarray(W2, np.float32), "b1b": b1b, "b2b": b2b,
            "dinvA": dinvA, "dinvO": dinvO[k], "iota": iota, "ident": ident,
            "idx1": layers[0]["idx"][k], "dst1": layers[0]["dst"][k],
            "idx2": layers[1]["idx"][k], "dst2": layers[1]["dst"][k],
        })
    meta = dict(L1=layers[0], L2=layers[1], k_of=k_of,
                pos128=w_of * 128 + dl_of)
    return in_maps, meta


def _emit_agg(nc, tc, meta_l, table, elem, elem_mm, acc, iota_sb, gbases,
              fin=None):
    """Aggregation phase: gather calls + one-hot matmuls + SBUF accumulate.
    elem = gathered element width (table columns); elem_mm = columns the
    matmul consumes (elem_mm <= elem). fin(w), if given, is emitted right
    after window w's last cell lands in the accumulator (progressive
    finalize, hidden under the remaining gather stream)."""
    calls, cells, chunk_cell = meta_l["calls"], meta_l["cells"], meta_l["chunk_cell"]
    last_cell_of_w = {}
    for ci_, (g_, w_, _, _) in enumerate(cells):
        last_cell_of_w[w_] = ci_
    fin_cells = {ci_: w_ for w_, ci_ in last_cell_of_w.items()}
    if fin is not None:
        assert len(last_cell_of_w) == acc.shape[1], \
            "every window needs at least one cell for progressive finalize"
    idx_d = table["idx"]
    dst_d = table["dst"]
    tbl = table["tbl"]
    with (
        tc.tile_pool(name=f"agg_sb_{elem_mm}", bufs=(6 if CALL <= 4096 else 3)) as sb2,
        tc.tile_pool(name=f"agg_oh_{elem_mm}", bufs=4) as ohp,
        tc.tile_pool(name=f"agg_ps_{elem_mm}", bufs=6, space="PSUM") as psp,
    ):
        cell_psum = {}
        cell_done = {}
        for ci, (g, s0, ns) in enumerate(calls):
            nch = ns // 128
            idx_t = sb2.tile([128, CALL // 16], I16, tag="idx")
            nc.sync.dma_start(out=idx_t[:, :ns // 16],
                              in_=idx_d[:, s0 // 16:(s0 + ns) // 16])
            dst_t = sb2.tile([128, CALL // 128], BF16, tag="dst")
            nc.sync.dma_start(out=dst_t[:, :nch],
                              in_=dst_d[:, s0 // 128:(s0 + ns) // 128])
            msg_t = sb2.tile([128, CALL // 128, elem], BF16, tag="msg")
            if os.environ.get("GCN_SKIP_GATHER"):
                nc.vector.memset(msg_t[:, :nch, :], 0.0)
            else:
                nc.gpsimd.dma_gather(
                    msg_t[:, :nch, :], tbl[gbases[g]:gbases[g + 1], :],
                    idx_t[:, :ns // 16], ns, ns, elem, elem_step=elem,
                    queue_num=ci % NQUEUES, single_packet=SINGLE_PACKET,
                )
            oh_tiles = []
            for h in range(0, nch, 32):
                hn = min(32, nch - h)
                oh = ohp.tile([128, 32, 128], BF16, tag="oh")
                nc.vector.tensor_tensor(
                    out=oh[:, :hn, :],
                    in0=dst_t[:, h:h + hn][:, :, None].to_broadcast([128, hn, 128]),
                    in1=iota_sb[:, :hn, :],
                    op=mybir.AluOpType.is_equal,
                )
                oh_tiles.append(oh)
            for lc in range(nch):
                gc = s0 // 128 + lc
                ci_cell = int(chunk_cell[gc])
                g_, w_, c0, nch_cell = cells[ci_cell]
                if ci_cell not in cell_psum:
                    cell_psum[ci_cell] = psp.tile([128, elem_mm], F32, tag="cps", name=f"cps{ci_cell}")
                    cell_done[ci_cell] = 0
                first = cell_done[ci_cell] == 0
                cell_done[ci_cell] += 1
                last = cell_done[ci_cell] == nch_cell
                nc.tensor.matmul(
                    out=cell_psum[ci_cell][:],
                    lhsT=oh_tiles[lc // 32][:, lc % 32, :],
                    rhs=msg_t[:, lc, :elem_mm],
                    start=first, stop=last,
                )
                if last:
                    nc.vector.tensor_tensor(
                        out=acc[:, w_, :], in0=acc[:, w_, :],
                        in1=cell_psum[ci_cell][:], op=mybir.AluOpType.add,
                    )
                    del cell_psum[ci_cell]
                    if fin is not None and ci_cell in fin_cells:
                        fin(fin_cells[ci_cell])


def build_program(cfg, meta):
    N, F1, F2 = cfg["N"], cfg["F1"], cfg["F2"]
    SHARD, NW, SHARD_PAD = cfg["SHARD"], cfg["NW"], cfg["SHARD_PAD"]
    ROWS, GW, XW = cfg["ROWS"], cfg["GW"], cfg["XW"]
    L1, L2 = meta["L1"], meta["L2"]
    gbases = [g * GW for g in range(NGROUPS)] + [ROWS]

    nc = bacc.Bacc(None, target_bir_lowering=False, debug=False,
                   num_swdge_queues=NQUEUES,
                   dynamic_dma_scratch_size=DMA_SCRATCH)
    xT_d = nc.dram_tensor("xT", [F1, ROWS], BF16, kind="ExternalInput")
    W1_d = nc.dram_tensor("W1", [F1, F1], BF16, kind="ExternalInput")
    W2_d = nc.dram_tensor("W2", [F1, F2], F32, kind="ExternalInput")
    b1b_d = nc.dram_tensor("b1b", [128, F1], F32, kind="ExternalInput")
    b2b_d = nc.dram_tensor("b2b", [128, F2], F32, kind="ExternalInput")
    dinvA_d = nc.dram_tensor("dinvA", [128, XW], F32, kind="ExternalInput")
    dinvO_d = nc.dram_tensor("dinvO", [128, NW], F32, kind="ExternalInput")
    iota_d = nc.dram_tensor("iota", [128, 32 * 128], BF16, kind="ExternalInput")
    ident_d = nc.dram_tensor("ident", [128, 128], F32, kind="ExternalInput")
    idx1_d = nc.dram_tensor("idx1", [128, L1["S"] // 16], I16, kind="ExternalInput")
    dst1_d = nc.dram_tensor("dst1", [128, L1["S"] // 128], BF16, kind="ExternalInput")
    idx2_d = nc.dram_tensor("idx2", [128, L2["S"] // 16], I16, kind="ExternalInput")
    dst2_d = nc.dram_tensor("dst2", [128, L2["S"] // 128], BF16, kind="ExternalInput")
    out_d = nc.dram_tensor("out", [SHARD_PAD, F2], F32, kind="ExternalOutput")

    # transposed bf16 g1 table: partition-major [128, XW*F1];
    # gather views it as [(p r) f] = [ROWS, F1] with row id sigma1.
    g1t_d = nc.dram_tensor("g1_tbl", [128, XW * F1], BF16)
    # locally-transposed bf16 g2 shard, feature-padded to 128 columns
    g2s_d = nc.dram_tensor("g2_shard", [128, NW * 128], BF16)
    g2f_d = nc.dram_tensor("g2_full", [NCORES * 128, NW * 128], BF16,
                           addr_space="Shared")
    g1_rows = g1t_d[:, :].rearrange("p (r f) -> (p r) f", f=F1)
    g2_rows = g2f_d[:, :].rearrange("c (w f) -> (c w) f", f=128)

    with tile.TileContext(nc) as tc:
        with tc.tile_pool(name="persist", bufs=1) as pp:
            w1_sb = pp.tile([F1, F1], BF16)
            nc.sync.dma_start(out=w1_sb[:], in_=W1_d[:, :])
            w2_sb = pp.tile([F1, F2], F32)
            nc.sync.dma_start(out=w2_sb[:], in_=W2_d[:, :])
            b1_sb = pp.tile([128, F1], F32)
            nc.sync.dma_start(out=b1_sb[:], in_=b1b_d[:, :])
            b2_sb = pp.tile([128, F2], F32)
            nc.sync.dma_start(out=b2_sb[:], in_=b2b_d[:, :])
            dinvA_sb = pp.tile([128, XW], F32)
            nc.sync.dma_start(out=dinvA_sb[:], in_=dinvA_d[:, :])
            dinvO_sb = pp.tile([128, NW], F32)
            nc.sync.dma_start(out=dinvO_sb[:], in_=dinvO_d[:, :])
            iota_sb = pp.tile([128, 32, 128], BF16)
            nc.sync.dma_start(out=iota_sb[:], in_=iota_d[:, :].rearrange("p (h d) -> p h d", d=128))
            ident_sb = pp.tile([128, 128], F32)
            nc.sync.dma_start(out=ident_sb[:], in_=ident_d[:, :])

            # ---- Phase A: g1 table (full, redundant per core) ----
            with (
                tc.tile_pool(name="pha_sb", bufs=3) as pa,
                tc.tile_pool(name="pha_ps", bufs=2, space="PSUM") as pap,
            ):
                for v0 in range(0, XW, PHA_WIN):
                    nwv = min(PHA_WIN, XW - v0)
                    xt = pa.tile([128, PHA_WIN * 128], BF16, tag="xt")
                    nc.sync.dma_start(
                        out=xt[:, :nwv * 128],
                        in_=xT_d[:, v0 * 128:(v0 + nwv) * 128])
                    hp = pap.tile([128, PHA_WIN * 128], F32, tag="hp")
                    for j in range(nwv):
                        nc.tensor.matmul(
                            out=hp[:, j * F1:(j + 1) * F1],
                            lhsT=xt[:, j * 128:(j + 1) * 128],
                            rhs=w1_sb[:], start=True, stop=True)
                    g1t = pa.tile([128, PHA_WIN, F1], BF16, tag="g1t")
                    hp3 = hp[:, :nwv * F1].rearrange("p (w f) -> p w f", w=nwv)
                    nc.vector.tensor_tensor(
                        out=g1t[:, :nwv, :],
                        in0=hp3,
                        in1=dinvA_sb[:, v0:v0 + nwv][:, :, None]
                            .to_broadcast([128, nwv, F1]),
                        op=mybir.AluOpType.mult)
                    # transposed table: partition p holds its own rows,
                    # consecutive windows are contiguous -> big descriptors
                    nc.sync.dma_start(
                        out=g1t_d[:, v0 * F1:(v0 + nwv) * F1]
                            .rearrange("p (w f) -> p w f", w=nwv),
                        in_=g1t[:, :nwv, :])

            # ---- Phase B: layer-1 aggregation ----
            with tc.tile_pool(name="acc1", bufs=1) as accp:
                acc = accp.tile([128, NW, F1], F32)
                # self-loop init: partition p's own-shard rows are contiguous
                # in the transposed table at dynamic offset pid*NW*F1.
                # SWDGE (gpsimd) DMA casts bf16 -> f32 on the way in.
                pid = nc.gpsimd.partition_id()
                self1 = g1t_d[:, 0:NW * F1].rearrange("p (w f) -> p w f", f=F1)
                self1 = bass.AP(self1.tensor, self1.offset + pid * (NW * F1),
                                self1.ap)
                nc.gpsimd.dma_start(out=acc[:], in_=self1)

                # ---- Phase C (progressive): layer-1 finalize + g2 shard ----
                # acc2 is allocated up front so the fp32 g2 windows can be
                # copied straight into it (layer-2 self-loop init). fin1(w)
                # runs as soon as window w's last cell lands, hidden under
                # the remaining layer-1 gather stream.
                with (
                    tc.tile_pool(name="acc2", bufs=1) as accp2,
                    tc.tile_pool(name="fin1", bufs=3) as fp,
                    tc.tile_pool(name="fin1_ps", bufs=1, space="PSUM") as fpp,
                    tc.tile_pool(name="fin1_ps2", bufs=1, space="PSUM") as fpp2,
                ):
                    acc2 = accp2.tile([128, NW, F2], F32)

                    def fin1(w):
                        # keep the DVE out of the finalize path where possible:
                        # per-partition-scalar muls and plain copies run on the
                        # (otherwise idle) scalar engine; only the per-feature
                        # bias add needs the DVE.
                        t = fp.tile([128, F1], F32, tag="t")
                        nc.scalar.mul(out=t[:], in_=acc[:, w, :],
                                      mul=dinvO_sb[:, w:w + 1])
                        nc.vector.tensor_tensor(
                            out=t[:], in0=t[:], in1=b1_sb[:],
                            op=mybir.AluOpType.add)
                        z = fp.tile([128, F1], F32, tag="z")
                        nc.scalar.activation(
                            out=z[:], in_=t[:],
                            func=mybir.ActivationFunctionType.Relu)
                        tp = fpp.tile([128, 128], F32, tag="tp")
                        nc.tensor.transpose(out=tp[:], in_=z[:],
                                            identity=ident_sb[:])
                        zT = fp.tile([128, F1], F32, tag="zT")
                        nc.scalar.copy(out=zT[:], in_=tp[:])
                        h2 = fpp2.tile([128, F2], F32, tag="h2")
                        nc.tensor.matmul(out=h2[:], lhsT=zT[:], rhs=w2_sb[:],
                                         start=True, stop=True)
                        g2t = fp.tile([128, F2], F32, tag="g2t")
                        nc.scalar.mul(out=g2t[:], in_=h2[:],
                                      mul=dinvO_sb[:, w:w + 1])
                        # layer-2 self-loop contribution
                        nc.scalar.copy(out=acc2[:, w, :], in_=g2t[:])
                        g2b = fp.tile([128, F2], BF16, tag="g2b")
                        nc.scalar.copy(out=g2b[:], in_=g2t[:])
                        nc.sync.dma_start(
                            out=g2s_d[:, w * 128:w * 128 + F2], in_=g2b[:])

                    # ---- Phase B: layer-1 aggregation (drives fin1) ----
                    _emit_agg(nc, tc, L1,
                              dict(idx=idx1_d, dst=dst1_d, tbl=g1_rows),
                              F1, F1, acc, iota_sb, gbases, fin=fin1)

                    # ---- AllGather g2 ----
                    nc.gpsimd.collective_compute(
                        "AllGather", mybir.AluOpType.bypass,
                        replica_groups=[list(range(NCORES))],
                        ins=[g2s_d.ap().opt()], outs=[g2f_d.ap().opt()])

                    # ---- Phase D/E: layer-2 aggregation + progressive out ----
                    with tc.tile_pool(name="fin2", bufs=3) as fp2:
                        def fin2(w):
                            o = fp2.tile([128, F2], F32, tag="o")
                            nc.scalar.mul(out=o[:], in_=acc2[:, w, :],
                                          mul=dinvO_sb[:, w:w + 1])
                            nc.vector.tensor_tensor(
                                out=o[:], in0=o[:], in1=b2_sb[:],
                                op=mybir.AluOpType.add)
                            nc.sync.dma_start(
                                out=out_d[w * 128:(w + 1) * 128, :], in_=o[:])

                        _emit_agg(nc, tc, L2,
                                  dict(idx=idx2_d, dst=dst2_d, tbl=g2_rows),
                                  128, F2, acc2, iota_sb, gbases, fin=fin2)

    nc.finalize()
    return nc


def _run(x, edge_index, W1, b1, W2, b2, n_nodes, trace=False):
    cfg = _cfg(n_nodes, int(W1.shape[1]), int(W2.shape[1]))
    in_maps, meta = _prep(x, edge_index, W1, b1, W2, b2, cfg)
    nc = build_program(cfg, meta)
    res = bass_utils.run_bass_kernel_spmd(
        nc, in_maps, core_ids=list(range(NCORES)), trace=trace)
    out = np.empty((n_nodes, cfg["F2"]), np.float32)
    for k in range(NCORES):
        m = meta["k_of"] == k
        out[m] = res.results[k]["out"][meta["pos128"][m]]
    return out, res


def kernel(x, edge_index, W1, b1, W2, b2):
    x = np.asarray(x)
    out, _ = _run(np.asarray(x, np.float32), np.asarray(edge_index),
                  np.asarray(W1, np.float32), np.asarray(b1, np.float32),
                  np.asarray(W2, np.float32), np.asarray(b2, np.float32),
                  n_nodes=x.shape[0])
    return out.astype(np.float32)


# revision 36
# speedup vs baseline: 1.0113x; 1.0113x over previous
"""2-layer GCN (PyG GCNConv semantics) on 8 Trainium2 NeuronCores.

Math: out = A_hat @ relu(A_hat @ X @ W1 + b1) @ W2 + b2,
      A_hat = D^-1/2 (A + I) D^-1/2, D = in-degree + 1.

Folding the symmetric norm into per-node scales:
  g = dinv * (H @ W);  s[c] = sum_{e:src->c} g[src] + g[c];  out = dinv*s + b

Sharding: destination-node ranges (12500 nodes/core). Each core:
  Phase A: compute the FULL g1 = dinv*(X@W1) table redundantly (PE idle
           anyway), stored BF16 and TRANSPOSED ([128, XW*F1] partition-major)
           so the writes are large contiguous descriptors.
  Phase B: batched dma_gather of edge-source rows (edges laid out in uniform
           (source-group, dest-window) cells so one SPMD program serves all
           cores), segment-sum via one-hot is_equal + bf16 matmul into PSUM
           per 128-dest window, accumulated into an fp32 SBUF accumulator.
           Self-loop contributions are NOT gathered: the accumulator is
           initialized from the core's own contiguous block of the transposed
           table (one register-offset SWDGE DMA with bf16->f32 cast).
  Phase C: finalize layer 1 per window, compute g2 shard (bf16, padded to
           128 feature columns so gather elements stay 256B-aligned); copy
           the fp32 g2 into the layer-2 accumulator (self-loop init for L2).
  AllGather g2 shards -> full g2 table.
  Phase D: layer-2 aggregation (same machinery; matmul consumes msg[:, :64]).
  Phase E: finalize layer 2, write output shard.

Gather descriptors are the dominant cost: ~250k edge slots per core per
layer, one 256B descriptor each. Descriptor generation runs on Q7 core pair
(2q, 2q+1) for SWDGE queue q, so rotating gather calls over 4 queues
parallelizes the emit 4x. The int16 gather-index limit (32767) forces
grouping edges by 25088-row source ranges; trailing chunk padding inside
each (window,group) cell uses src_local=0 and dest_local=-1 (one-hot zero).
"""

import os
import numpy as np
import ml_dtypes

import concourse.bass as bass
from concourse import bacc
import concourse.mybir as mybir
import concourse.tile as tile
from concourse import bass_utils

F32 = mybir.dt.float32
BF16 = mybir.dt.bfloat16
I16 = mybir.dt.int16
NPBF16 = ml_dtypes.bfloat16

NCORES = 8
NGROUPS = 4
CALL = int(os.environ.get('GCN_CALL', '4096'))   # gather-call size in edge slots
NQUEUES = int(os.environ.get('GCN_NQUEUES', '4'))
DMA_SCRATCH = int(os.environ.get('GCN_DMA_SCRATCH', '32768'))
SINGLE_PACKET = bool(int(os.environ.get('GCN_SP', '0')))
PHA_WIN = 16         # windows per phase-A iteration


def _cfg(n_nodes, f1, f2):
    shard = n_nodes // NCORES
    nw = (shard + 127) // 128
    shard_pad = nw * 128
    rows = NCORES * shard_pad          # padded table rows (both layers)
    gw = -(-rows // NGROUPS)
    gw = ((gw + 127) // 128) * 128     # group width, multiple of 128
    assert gw <= 32768, gw
    return dict(N=n_nodes, F1=f1, F2=f2, SHARD=shard, NW=nw,
                SHARD_PAD=shard_pad, ROWS=rows, GW=gw,
                XW=rows // 128)


def _layout(cnt_kwg, nw):
    """cnt_kwg: [NCORES, NW, NGROUPS] edge counts. Returns uniform cell
    capacities C[g][w] (multiples of 128), cell slot offsets, total slots S,
    gather call list [(g, slot0, nslots)], and per-chunk cell bookkeeping."""
    cmax = cnt_kwg.max(axis=0)                      # [NW, NGROUPS]
    C = ((cmax + 127) // 128) * 128                 # [NW, NGROUPS]
    cell_off = np.zeros((NGROUPS, nw), np.int64)
    off = 0
    calls = []
    cells = []            # list of (g, w, chunk0, nchunks)
    chunk_cell = []       # per global chunk -> cell index
    for g in range(NGROUPS):
        g0 = off
        for w in range(nw):
            cell_off[g, w] = off
            c = int(C[w, g])
            if c > 0:
                ci = len(cells)
                cells.append((g, w, off // 128, c // 128))
                chunk_cell.extend([ci] * (c // 128))
            off += c
        glen = off - g0
        s = g0
        while s < off:
            ns = min(CALL, off - s)
            calls.append((g, s, ns))
            s += ns
        assert glen % 128 == 0
    return C, cell_off, off, calls, cells, np.asarray(chunk_cell)


def _pack_edges(slot, lidx, dl, S):
    """Build the wrapped int16 index tensor and packed dest-local tensor."""
    src = np.zeros(S, np.int16)
    dst = np.full(S, -1.0, np.float32)
    src[slot] = lidx.astype(np.int16)
    dst[slot] = dl.astype(np.float32)
    idx_w = np.tile(src.reshape(S // 16, 16).T, (8, 1)).copy()    # [128, S/16]
    dst_p = dst.reshape(S // 128, 128).T.astype(NPBF16)           # [128, S/128]
    return idx_w, dst_p


def _prep(x, edge_index, W1, b1, W2, b2, cfg):
    N, SHARD, NW, SHARD_PAD, GW = (cfg["N"], cfg["SHARD"], cfg["NW"],
                                   cfg["SHARD_PAD"], cfg["GW"])
    ROWS, F1, F2, XW = cfg["ROWS"], cfg["F1"], cfg["F2"], cfg["XW"]

    row = np.asarray(edge_index[0]).astype(np.int64)
    col = np.asarray(edge_index[1]).astype(np.int64)
    deg = (np.bincount(col, minlength=N) + 1).astype(np.float32)
    dinv = (1.0 / np.sqrt(deg)).astype(np.float32)

    # balanced destination assignment: nodes sorted by in-degree, dealt
    # round-robin over cores, filling windows sequentially. Equalizes per
    # (window, source-group) cell counts across cores, shrinking the
    # max-over-cores cell padding (~5% fewer gather slots).
    indeg = np.bincount(col, minlength=N)
    order = np.argsort(-indeg, kind='stable')
    ranks = np.empty(N, np.int64)
    ranks[order] = np.arange(N)
    k_of = ranks % NCORES
    pos = ranks // NCORES
    w_of = pos >> 7
    dl_of = pos & 127
    rho_of = k_of * SHARD_PAD + pos  # pos < SHARD <= SHARD_PAD

    # padded node order: node n -> padded row rho_of[n]
    dinv_pad = np.zeros(ROWS, np.float32)
    xT_pad = np.zeros((F1, ROWS), NPBF16)
    xT = np.ascontiguousarray(np.asarray(x, np.float32).T.astype(NPBF16))
    dinv_pad[rho_of] = dinv
    xT_pad[:, rho_of] = xT

    k_arr = k_of[col]
    wl = w_of[col]
    dl = dl_of[col]

    # layer-1 source ids: sigma1 over the transposed padded g1 table
    # (padded row rho -> table row (rho%128)*XW + rho//128).
    rho = rho_of[row]
    src1 = (rho % 128) * XW + rho // 128
    # layer-2: g2f is a stack of locally-transposed shards [NCORES*128, NW*128]
    # (bf16, feature-padded); node at (k, w, p) -> row (k*128 + p)*NW + w.
    src2 = (k_of[row] * 128 + dl_of[row]) * NW + w_of[row]

    layers = []
    for src in (src1, src2):
        g = src // GW
        lidx = src - g * GW
        key = (k_arr * NW + wl) * NGROUPS + g
        cnt = np.bincount(key, minlength=NCORES * NW * NGROUPS)
        cnt = cnt.reshape(NCORES, NW, NGROUPS)
        C, cell_off, S, calls, cells, chunk_cell = _layout(cnt, NW)
        order = np.lexsort((wl, g, k_arr))
        ks, gs, ws = k_arr[order], g[order], wl[order]
        keys = (ks * NGROUPS + gs) * NW + ws
        starts = np.r_[0, np.nonzero(np.diff(keys))[0] + 1]
        run_id = np.zeros(len(keys), np.int64)
        run_id[starts[1:]] = 1
        run_id = np.cumsum(run_id)
        within = np.arange(len(keys)) - starts[run_id]
        slot = cell_off[gs, ws] + within
        idx_list, dst_list = [], []
        for k in range(NCORES):
            m = ks == k
            iw, dp = _pack_edges(slot[m], lidx[order][m], dl[order][m], S)
            idx_list.append(iw)
            dst_list.append(dp)
        layers.append(dict(S=S, calls=calls, cells=cells,
                           chunk_cell=chunk_cell, idx=idx_list, dst=dst_list))

    dinvA = dinv_pad.reshape(XW, 128).T.copy()                 # [128, XW]
    dinvO = [dinv_pad[k * SHARD_PAD:(k + 1) * SHARD_PAD].reshape(NW, 128).T.copy()
             for k in range(NCORES)]
    iota = np.tile(np.arange(128, dtype=np.float32), (128, 32, 1)).astype(NPBF16)
    ident = np.eye(128, dtype=np.float32)
    b1b = np.tile(np.asarray(b1, np.float32), (128, 1))
    b2b = np.tile(np.asarray(b2, np.float32), (128, 1))

    in_maps = []
    for k in range(NCORES):
        in_maps.append({
            "xT": xT_pad, "W1": np.asarray(W1, np.float32).astype(NPBF16),
            "W2": np.asarray(W2, np.float32), "b1b": b1b, "b2b": b2b,
            "dinvA": dinvA, "dinvO": dinvO[k], "iota": iota, "ident": ident,
            "idx1": layers[0]["idx"][k], "dst1": layers[0]["dst"][k],
            "idx2": layers[1]["idx"][k], "dst2": layers[1]["dst"][k],
        })
    meta = dict(L1=layers[0], L2=layers[1], k_of=k_of,
                pos128=w_of * 128 + dl_of)
    return in_maps, meta


def _emit_agg(nc, tc, meta_l, table, elem, elem_mm, acc, iota_sb, gbases,
              fin=None):
    """Aggregation phase: gather calls + one-hot matmuls + SBUF accumulate.
    elem = gathered element width (table columns); elem_mm = columns the
    matmul consumes (elem_mm <= elem). fin(w), if given, is emitted right
    after window w's last cell lands in the accumulator (progressive
    finalize, hidden under the remaining gather stream)."""
    calls, cells, chunk_cell = meta_l["calls"], meta_l["cells"], meta_l["chunk_cell"]
    last_cell_of_w = {}
    for ci_, (g_, w_, _, _) in enumerate(cells):
        last_cell_of_w[w_] = ci_
    fin_cells = {ci_: w_ for w_, ci_ in last_cell_of_w.items()}
    if fin is not None:
        assert len(last_cell_of_w) == acc.shape[1], \
            "every window needs at least one cell for progressive finalize"
    idx_d = table["idx"]
    dst_d = table["dst"]
    tbl = table["tbl"]
    with (
        tc.tile_pool(name=f"agg_sb_{elem_mm}", bufs=(6 if CALL <= 4096 else 3)) as sb2,
        tc.tile_pool(name=f"agg_oh_{elem_mm}", bufs=3) as ohp,
        tc.tile_pool(name=f"agg_ps_{elem_mm}", bufs=6, space="PSUM") as psp,
    ):
        cell_psum = {}
        cell_done = {}
        for ci, (g, s0, ns) in enumerate(calls):
            nch = ns // 128
            idx_t = sb2.tile([128, CALL // 16], I16, tag="idx")
            nc.sync.dma_start(out=idx_t[:, :ns // 16],
                              in_=idx_d[:, s0 // 16:(s0 + ns) // 16])
            dst_t = sb2.tile([128, CALL // 128], BF16, tag="dst")
            nc.sync.dma_start(out=dst_t[:, :nch],
                              in_=dst_d[:, s0 // 128:(s0 + ns) // 128])
            msg_t = sb2.tile([128, CALL // 128, elem], BF16, tag="msg")
            if os.environ.get("GCN_SKIP_GATHER"):
                nc.vector.memset(msg_t[:, :nch, :], 0.0)
            else:
                nc.gpsimd.dma_gather(
                    msg_t[:, :nch, :], tbl[gbases[g]:gbases[g + 1], :],
                    idx_t[:, :ns // 16], ns, ns, elem, elem_step=elem,
                    queue_num=((ci % 2) * 2 + (ci // 2) % 2) % NQUEUES,
                    single_packet=SINGLE_PACKET,
                )
            oh_tiles = []
            for h in range(0, nch, 32):
                hn = min(32, nch - h)
                oh = ohp.tile([128, 32, 128], BF16, tag="oh")
                nc.vector.tensor_tensor(
                    out=oh[:, :hn, :],
                    in0=dst_t[:, h:h + hn][:, :, None].to_broadcast([128, hn, 128]),
                    in1=iota_sb[:, :hn, :],
                    op=mybir.AluOpType.is_equal,
                )
                oh_tiles.append(oh)
            for lc in range(nch):
                gc = s0 // 128 + lc
                ci_cell = int(chunk_cell[gc])
                g_, w_, c0, nch_cell = cells[ci_cell]
                if ci_cell not in cell_psum:
                    cell_psum[ci_cell] = psp.tile([128, elem_mm], F32, tag="cps", name=f"cps{ci_cell}")
                    cell_done[ci_cell] = 0
                first = cell_done[ci_cell] == 0
                cell_done[ci_cell] += 1
                last = cell_done[ci_cell] == nch_cell
                nc.tensor.matmul(
                    out=cell_psum[ci_cell][:],
                    lhsT=oh_tiles[lc // 32][:, lc % 32, :],
                    rhs=msg_t[:, lc, :elem_mm],
                    start=first, stop=last,
                )
                if last:
                    nc.vector.tensor_tensor(
                        out=acc[:, w_, :], in0=acc[:, w_, :],
                        in1=cell_psum[ci_cell][:], op=mybir.AluOpType.add,
                    )
                    del cell_psum[ci_cell]
                    if fin is not None and ci_cell in fin_cells:
                        fin(fin_cells[ci_cell])


def build_program(cfg, meta):
    N, F1, F2 = cfg["N"], cfg["F1"], cfg["F2"]
    SHARD, NW, SHARD_PAD = cfg["SHARD"], cfg["NW"], cfg["SHARD_PAD"]
    ROWS, GW, XW = cfg["ROWS"], cfg["GW"], cfg["XW"]
    L1, L2 = meta["L1"], meta["L2"]
    gbases = [g * GW for g in range(NGROUPS)] + [ROWS]

    nc = bacc.Bacc(None, target_bir_lowering=False, debug=False,
                   num_swdge_queues=NQUEUES,
                   dynamic_dma_scratch_size=DMA_SCRATCH)
    xT_d = nc.dram_tensor("xT", [F1, ROWS], BF16, kind="ExternalInput")
    W1_d = nc.dram_tensor("W1", [F1, F1], BF16, kind="ExternalInput")
    W2_d = nc.dram_tensor("W2", [F1, F2], F32, kind="ExternalInput")
    b1b_d = nc.dram_tensor("b1b", [128, F1], F32, kind="ExternalInput")
    b2b_d = nc.dram_tensor("b2b", [128, F2], F32, kind="ExternalInput")
    dinvA_d = nc.dram_tensor("dinvA", [128, XW], F32, kind="ExternalInput")
    dinvO_d = nc.dram_tensor("dinvO", [128, NW], F32, kind="ExternalInput")
    iota_d = nc.dram_tensor("iota", [128, 32 * 128], BF16, kind="ExternalInput")
    ident_d = nc.dram_tensor("ident", [128, 128], F32, kind="ExternalInput")
    idx1_d = nc.dram_tensor("idx1", [128, L1["S"] // 16], I16, kind="ExternalInput")
    dst1_d = nc.dram_tensor("dst1", [128, L1["S"] // 128], BF16, kind="ExternalInput")
    idx2_d = nc.dram_tensor("idx2", [128, L2["S"] // 16], I16, kind="ExternalInput")
    dst2_d = nc.dram_tensor("dst2", [128, L2["S"] // 128], BF16, kind="ExternalInput")
    out_d = nc.dram_tensor("out", [SHARD_PAD, F2], F32, kind="ExternalOutput")

    # transposed bf16 g1 table: partition-major [128, XW*F1];
    # gather views it as [(p r) f] = [ROWS, F1] with row id sigma1.
    g1t_d = nc.dram_tensor("g1_tbl", [128, XW * F1], BF16)
    # locally-transposed bf16 g2 shard, feature-padded to 128 columns
    g2s_d = nc.dram_tensor("g2_shard", [128, NW * 128], BF16)
    g2f_d = nc.dram_tensor("g2_full", [NCORES * 128, NW * 128], BF16,
                           addr_space="Shared")
    g1_rows = g1t_d[:, :].rearrange("p (r f) -> (p r) f", f=F1)
    g2_rows = g2f_d[:, :].rearrange("c (w f) -> (c w) f", f=128)

    with tile.TileContext(nc) as tc:
        with tc.tile_pool(name="persist", bufs=1) as pp:
            w1_sb = pp.tile([F1, F1], BF16)
            nc.sync.dma_start(out=w1_sb[:], in_=W1_d[:, :])
            w2_sb = pp.tile([F1, F2], F32)
            nc.sync.dma_start(out=w2_sb[:], in_=W2_d[:, :])
            b1_sb = pp.tile([128, F1], F32)
            nc.sync.dma_start(out=b1_sb[:], in_=b1b_d[:, :])
            b2_sb = pp.tile([128, F2], F32)
            nc.sync.dma_start(out=b2_sb[:], in_=b2b_d[:, :])
            dinvA_sb = pp.tile([128, XW], F32)
            nc.sync.dma_start(out=dinvA_sb[:], in_=dinvA_d[:, :])
            dinvO_sb = pp.tile([128, NW], F32)
            nc.sync.dma_start(out=dinvO_sb[:], in_=dinvO_d[:, :])
            iota_sb = pp.tile([128, 32, 128], BF16)
            nc.sync.dma_start(out=iota_sb[:], in_=iota_d[:, :].rearrange("p (h d) -> p h d", d=128))
            ident_sb = pp.tile([128, 128], F32)
            nc.sync.dma_start(out=ident_sb[:], in_=ident_d[:, :])

            # ---- Phase A: g1 table (full, redundant per core) ----
            with (
                tc.tile_pool(name="pha_sb", bufs=3) as pa,
                tc.tile_pool(name="pha_ps", bufs=2, space="PSUM") as pap,
            ):
                for v0 in range(0, XW, PHA_WIN):
                    nwv = min(PHA_WIN, XW - v0)
                    xt = pa.tile([128, PHA_WIN * 128], BF16, tag="xt")
                    nc.sync.dma_start(
                        out=xt[:, :nwv * 128],
                        in_=xT_d[:, v0 * 128:(v0 + nwv) * 128])
                    hp = pap.tile([128, PHA_WIN * 128], F32, tag="hp")
                    for j in range(nwv):
                        nc.tensor.matmul(
                            out=hp[:, j * F1:(j + 1) * F1],
                            lhsT=xt[:, j * 128:(j + 1) * 128],
                            rhs=w1_sb[:], start=True, stop=True)
                    g1t = pa.tile([128, PHA_WIN, F1], BF16, tag="g1t")
                    hp3 = hp[:, :nwv * F1].rearrange("p (w f) -> p w f", w=nwv)
                    nc.vector.tensor_tensor(
                        out=g1t[:, :nwv, :],
                        in0=hp3,
                        in1=dinvA_sb[:, v0:v0 + nwv][:, :, None]
                            .to_broadcast([128, nwv, F1]),
                        op=mybir.AluOpType.mult)
                    # transposed table: partition p holds its own rows,
                    # consecutive windows are contiguous -> big descriptors
                    nc.sync.dma_start(
                        out=g1t_d[:, v0 * F1:(v0 + nwv) * F1]
                            .rearrange("p (w f) -> p w f", w=nwv),
                        in_=g1t[:, :nwv, :])

            # ---- Phase B: layer-1 aggregation ----
            with tc.tile_pool(name="acc1", bufs=1) as accp:
                acc = accp.tile([128, NW, F1], F32)
                # self-loop init: partition p's own-shard rows are contiguous
                # in the transposed table at dynamic offset pid*NW*F1.
                # SWDGE (gpsimd) DMA casts bf16 -> f32 on the way in.
                pid = nc.gpsimd.partition_id()
                self1 = g1t_d[:, 0:NW * F1].rearrange("p (w f) -> p w f", f=F1)
                self1 = bass.AP(self1.tensor, self1.offset + pid * (NW * F1),
                                self1.ap)
                nc.gpsimd.dma_start(out=acc[:], in_=self1)

                # ---- Phase C (progressive): layer-1 finalize + g2 shard ----
                # acc2 is allocated up front so the fp32 g2 windows can be
                # copied straight into it (layer-2 self-loop init). fin1(w)
                # runs as soon as window w's last cell lands, hidden under
                # the remaining layer-1 gather stream.
                with (
                    tc.tile_pool(name="acc2", bufs=1) as accp2,
                    tc.tile_pool(name="fin1", bufs=3) as fp,
                    tc.tile_pool(name="fin1_ps", bufs=1, space="PSUM") as fpp,
                    tc.tile_pool(name="fin1_ps2", bufs=1, space="PSUM") as fpp2,
                ):
                    acc2 = accp2.tile([128, NW, F2], F32)

                    def fin1(w):
                        # keep the DVE out of the finalize path where possible:
                        # per-partition-scalar muls and plain copies run on the
                        # (otherwise idle) scalar engine; only the per-feature
                        # bias add needs the DVE.
                        t = fp.tile([128, F1], F32, tag="t")
                        nc.scalar.mul(out=t[:], in_=acc[:, w, :],
                                      mul=dinvO_sb[:, w:w + 1])
                        nc.vector.tensor_tensor(
                            out=t[:], in0=t[:], in1=b1_sb[:],
                            op=mybir.AluOpType.add)
                        z = fp.tile([128, F1], F32, tag="z")
                        nc.scalar.activation(
                            out=z[:], in_=t[:],
                            func=mybir.ActivationFunctionType.Relu)
                        tp = fpp.tile([128, 128], F32, tag="tp")
                        nc.tensor.transpose(out=tp[:], in_=z[:],
                                            identity=ident_sb[:])
                        zT = fp.tile([128, F1], F32, tag="zT")
                        nc.scalar.copy(out=zT[:], in_=tp[:])
                        h2 = fpp2.tile([128, F2], F32, tag="h2")
                        nc.tensor.matmul(out=h2[:], lhsT=zT[:], rhs=w2_sb[:],
                                         start=True, stop=True)
                        g2t = fp.tile([128, F2], F32, tag="g2t")
                        nc.scalar.mul(out=g2t[:], in_=h2[:],
                                      mul=dinvO_sb[:, w:w + 1])
                        # layer-2 self-loop contribution
                        nc.scalar.copy(out=acc2[:, w, :], in_=g2t[:])
                        g2b = fp.tile([128, F2], BF16, tag="g2b")
                        nc.scalar.copy(out=g2b[:], in_=g2t[:])
                        nc.sync.dma_start(
                            out=g2s_d[:, w * 128:w * 128 + F2], in_=g2b[:])

                    # ---- Phase B: layer-1 aggregation (drives fin1) ----
                    _emit_agg(nc, tc, L1,
                              dict(idx=idx1_d, dst=dst1_d, tbl=g1_rows),
                              F1, F1, acc, iota_sb, gbases, fin=fin1)

                    # ---- AllGather g2 ----
                    nc.gpsimd.collective_compute(
                        "AllGather", mybir.AluOpType.bypass,
                        replica_groups=[list(range(NCORES))],
                        ins=[g2s_d.ap().opt()], outs=[g2f_d.ap().opt()])

                    # ---- Phase D/E: layer-2 aggregation + progressive out ----
                    with tc.tile_pool(name="fin2", bufs=3) as fp2:
                        def fin2(w):
                            o = fp2.tile([128, F2], F32, tag="o")
                            nc.scalar.mul(out=o[:], in_=acc2[:, w, :],
                                          mul=dinvO_sb[:, w:w + 1])
                            nc.vector.tensor_tensor(
                                out=o[:], in0=o[:], in1=b2_sb[:],
                                op=mybir.AluOpType.add)
                            nc.sync.dma_start(
                                out=out_d[w * 128:(w + 1) * 128, :], in_=o[:])

                        _emit_agg(nc, tc, L2,
                                  dict(idx=idx2_d, dst=dst2_d, tbl=g2_rows),
                                  128, F2, acc2, iota_sb, gbases, fin=fin2)

    nc.finalize()
    return nc


def _run(x, edge_index, W1, b1, W2, b2, n_nodes, trace=False):
    cfg = _cfg(n_nodes, int(W1.shape[1]), int(W2.shape[1]))
    in_maps, meta = _prep(x, edge_index, W1, b1, W2, b2, cfg)
    nc = build_program(cfg, meta)
    res = bass_utils.run_bass_kernel_spmd(
        nc, in_maps, core_ids=list(range(NCORES)), trace=trace)
    out = np.empty((n_nodes, cfg["F2"]), np.float32)
    for k in range(NCORES):
        m = meta["k_of"] == k
        out[m] = res.results[k]["out"][meta["pos128"][m]]
    return out, res


def kernel(x, edge_index, W1, b1, W2, b2):
    x = np.asarray(x)
    out, _ = _run(np.asarray(x, np.float32), np.asarray(edge_index),
                  np.asarray(W1, np.float32), np.asarray(b1, np.float32),
                  np.asarray(W2, np.float32), np.asarray(b2, np.float32),
                  n_nodes=x.shape[0])
    return out.astype(np.float32)
